# revision 27
# baseline (speedup 1.0000x reference)
"""Trainium2 Bass kernel for 3-layer GraphSAGE (nn_MCHCGraphSage).

Strategy (8 NeuronCores, SPMD single program):
  - Destination-sharded edges: core k owns dst nodes [k*6250, (k+1)*6250).
  - Node features stored in HBM as 256B rows in TWO half-spaces split by
    local row: half1 = slab rows [0, 3200), half2 = [3200, 6272).  Row
    addresses then fit int16 for the gpsimd dma_gather (25600 / 24576 rows).
  - Per batch of 4 dst windows, FOUR dma_gathers (half1/half2 x 2) on SWDGE
    queues 1,2,3,0: each queue uses its own Q7 core pair, so descriptor
    generation runs 4-way parallel (queue 0 dispatch blocks, so it goes
    last).
  - Aggregation: single-level one-hot matmul.  For each 128-slot chunk,
    matmul(win_ps[64, 128dst], lhsT=chunk[128slots, 64feat],
    rhs=oh[128slots, 128dst]) accumulates the *mean* directly: the host
    bakes 1/deg into the one-hot values.  Pad slots have all-zero one-hot
    rows, so no degree padding and no zero-fill matmuls are needed.
    One-hots stream from HBM per batch (double buffered).
  - Dense part per window, node-major: y = meanT.T @ Wl + hselfT.T @ Ws_ext
    (bias folded as an extra ones-row of hselfT), ReLU on ACT, PE-transpose
    to keep the feature-major self slab for the next layer.
  - Inter-layer redistribution: TWO partial AllGathers per layer (half1
    fires as soon as windows 0-24 are done, overlapping the rest of the
    layer; half2 at layer end), into per-layer-parity hext buffers.
"""

import os
import sys

import numpy as np

for _p in ("/opt/trn_rl_repo", "/root/.axon_site/_ro/trn_rl_repo"):
    if os.path.isdir(_p) and _p not in sys.path:
        sys.path.append(_p)

import ml_dtypes  # noqa: E402

N = 50000
D = 64
NCORES = 8
SLAB = 6250
PSLAB = 6272
WIN = 128
NW = PSLAB // WIN  # 49
HALF = 3200        # local rows in half1 (windows 0-24)
H2 = PSLAB - HALF  # 3072 (windows 25-48)
NW1 = HALF // WIN  # 25
H1TOT = NCORES * HALF   # 25600
H2TOT = NCORES * H2     # 24576
AZERO = H1TOT           # appended all-zero row in xext1/hext1
BZERO = 3050            # core-0 slab pad rows (local 6250) are always zero
BW = 4  # windows per gather batch

_NC_CACHE = {}
LAST_RESULTS = None  # test harness introspection (exec_time_ns, profile)


def _pack(x, edge_index, scale):
    """Host-side packing. Returns per-core dicts + structure constants."""
    src = np.asarray(edge_index[0], dtype=np.int64)
    dst = np.asarray(edge_index[1], dtype=np.int64)
    k_src = src // SLAB
    loc = src % SLAB
    isA_e = loc < HALF
    rowA_e = k_src * HALF + loc
    rowB_e = k_src * H2 + (loc - HALF)

    # pass 1: section sizes
    nch_a = 0
    nch_b = 0
    per_core = []
    for k in range(NCORES):
        sel = (dst >= k * SLAB) & (dst < (k + 1) * SLAB)
        d_k = dst[sel] - k * SLAB
        isA = isA_e[sel]
        row_k = np.where(isA, rowA_e[sel], rowB_e[sel])
        degA = np.bincount(d_k[isA], minlength=PSLAB)
        degB = np.bincount(d_k[~isA], minlength=PSLAB)
        wA = degA.reshape(NW, WIN).sum(1).max()
        wB = degB.reshape(NW, WIN).sum(1).max()
        nch_a = max(nch_a, (int(wA) + 127) // 128)
        nch_b = max(nch_b, (int(wB) + 127) // 128)
        per_core.append((d_k, row_k, isA, degA, degB))

    S_A = nch_a * 128
    S_B = nch_b * 128
    NCH = nch_a + nch_b
    T_A = NW * S_A
    T_B = NW * S_B
    fdt = ml_dtypes.bfloat16
    ROW = 128

    # node features in the two half-spaces (256B rows, 64 used)
    nodes = np.arange(N)
    nloc = nodes % SLAB
    nk = nodes // SLAB
    xext1 = np.zeros((H1TOT + 1, ROW), dtype=fdt)
    xext2 = np.zeros((H2TOT, ROW), dtype=fdt)
    m1 = nloc < HALF
    xext1[(nk * HALF + nloc)[m1], :D] = x[m1].astype(fdt)
    xext2[(nk * H2 + nloc - HALF)[~m1], :D] = x[~m1].astype(fdt)

    cores = []
    for k in range(NCORES):
        d_k, row_k, isA, degA, degB = per_core[k]
        offA = degA.reshape(NW, WIN)
        offA = (np.cumsum(offA, 1) - offA).reshape(-1)
        offB = degB.reshape(NW, WIN)
        offB = (np.cumsum(offB, 1) - offB).reshape(-1)

        def build(mask, deg, off, S, padval):
            e_d = d_k[mask]
            e_r = row_k[mask]
            order = np.argsort(e_d, kind="stable")
            d_s = e_d[order]
            r_s = e_r[order]
            start = np.concatenate([[0], np.cumsum(deg)])[:-1]
            rank = np.arange(len(d_s)) - start[d_s]
            pos = (d_s // WIN) * S + off[d_s] + rank
            stream = np.full(NW * S, padval, dtype=np.int64)
            stream[pos] = r_s
            return stream, pos, d_s

        streamA, posA, dA = build(isA, degA, offA, S_A, AZERO)
        streamB, posB, dB = build(~isA, degB, offB, S_B, BZERO)
        assert streamA.max() <= AZERO and streamB.max() < H2TOT
        assert streamA.min() >= 0 and streamB.min() >= 0

        # per-slot dst code + 1/deg scale; the device builds the one-hot
        # rhs per chunk as (iota == code) * scale on the DVE.
        code = np.full((128, NW * NCH), 999.0, dtype=fdt)
        sc_k = scale[k * SLAB : (k + 1) * SLAB]
        scp = np.zeros(PSLAB, dtype=np.float32)
        scp[:SLAB] = sc_k
        for pos, d_s, cc0, S in ((posA, dA, 0, S_A), (posB, dB, nch_a, S_B)):
            w = pos // S
            r = pos % S
            cc = cc0 + r // 128
            prow = r % 128
            code[prow, w * NCH + cc] = (d_s % WIN).astype(np.float32)
        scl2 = np.tile(scp.astype(np.float32), (D, 1)).astype(fdt)

        stream = np.concatenate([streamA, streamB]).astype(np.int16)
        idx16 = stream.reshape(-1, 16).T.copy()  # [16, T/16]
        idx = np.tile(idx16, (8, 1))  # replicate for 8 gpsimd cores

        xselfT = np.zeros((D + 1, PSLAB), dtype=fdt)
        xselfT[:D, :SLAB] = x[k * SLAB : (k + 1) * SLAB].T.astype(fdt)
        xselfT[D, :] = 1.0  # bias row

        cores.append({"idx": idx, "code": code, "scl2": scl2, "xselfT": xselfT})

    return nch_a, nch_b, xext1, xext2, cores


def _build_nc(nch_a, nch_b):
    import concourse.bacc as bacc
    import concourse.tile as tile
    import concourse.mybir as mybir

    dt = mybir.dt
    fdt = dt.bfloat16
    ROW = 128
    NCH = nch_a + nch_b
    S_A = nch_a * 128
    S_B = nch_b * 128
    T_A = NW * S_A
    T_B = NW * S_B

    nqueues = int(os.environ.get("SAGE_QUEUES", "4"))
    nc = bacc.Bacc(None, num_devices=NCORES, num_swdge_queues=nqueues)

    xe1_d = nc.dram_tensor("xext1", [H1TOT + 1, ROW], fdt, kind="ExternalInput")
    xe2_d = nc.dram_tensor("xext2", [H2TOT, ROW], fdt, kind="ExternalInput")
    idx_d = nc.dram_tensor(
        "idx", [128, (T_A + T_B) // 16], dt.int16, kind="ExternalInput"
    )
    code_d = nc.dram_tensor(
        "code", [128, NW * NCH], dt.bfloat16, kind="ExternalInput"
    )
    scl2_d = nc.dram_tensor(
        "scl2", [D, PSLAB], dt.bfloat16, kind="ExternalInput"
    )
    iota_d = nc.dram_tensor("iota", [128, 128], dt.bfloat16, kind="ExternalInput")
    xsT_d = nc.dram_tensor("xselfT", [D + 1, PSLAB], fdt, kind="ExternalInput")
    ident_d = nc.dram_tensor("ident", [WIN, WIN], fdt, kind="ExternalInput")
    w_d = {}
    for l, m in ((0, D), (1, D), (2, 1)):
        w_d[f"wl{l}"] = nc.dram_tensor(f"wl{l}", [D, m], fdt, kind="ExternalInput")
        w_d[f"ws{l}"] = nc.dram_tensor(
            f"ws{l}", [D + 1, m], fdt, kind="ExternalInput"
        )
    out_d = nc.dram_tensor("out", [PSLAB, 1], dt.float32, kind="ExternalOutput")

    he1 = [nc.dram_tensor(f"hext1{p}", [H1TOT + 1, ROW], fdt, addr_space="Shared")
           for p in "ab"]
    he2 = [nc.dram_tensor(f"hext2{p}", [H2TOT, ROW], fdt, addr_space="Shared")
           for p in "ab"]
    slab1_d = nc.dram_tensor("slab1", [HALF, ROW], fdt)
    slab2_d = nc.dram_tensor("slab2", [H2, ROW], fdt)

    batches = []
    w0 = 0
    while w0 < NW:
        bw = min(BW, NW - w0)
        batches.append((w0, bw))
        w0 += bw

    with tile.TileContext(nc) as tc:
        with (
            tc.tile_pool(name="const", bufs=1) as cpool,
            tc.tile_pool(name="gpool", bufs=2) as gpool,
            tc.tile_pool(name="ohpool", bufs=2) as ohpool,
            tc.tile_pool(name="spool", bufs=4) as spool,
            tc.tile_pool(name="psA", bufs=4, space="PSUM") as psA,
            tc.tile_pool(name="psB", bufs=2, space="PSUM") as psB,
            tc.tile_pool(name="psC", bufs=2, space="PSUM") as psC,
        ):
            idx_sb = cpool.tile([128, (T_A + T_B) // 16], dt.int16, tag="idx")
            code_sb = cpool.tile([128, NW * NCH], dt.bfloat16, tag="code")
            scl2_sb = cpool.tile([D, PSLAB], dt.bfloat16, tag="scl2")
            iota_sb = cpool.tile([128, 128], dt.bfloat16, tag="iota")
            ident_sb = cpool.tile([WIN, WIN], fdt, tag="ident")
            zrow_sb = cpool.tile([1, ROW], fdt, tag="zrow")
            zpad_sb = cpool.tile([PSLAB - SLAB, ROW], fdt, tag="zpad")
            hs = [cpool.tile([D + 1, PSLAB], fdt, tag=f"hs{i}", name=f"hs{i}")
                  for i in range(3)]
            w_sb = {}
            for l, m in ((0, D), (1, D), (2, 1)):
                w_sb[f"wl{l}"] = cpool.tile([D, m], fdt, tag=f"wl{l}",
                                            name=f"wl{l}")
                w_sb[f"ws{l}"] = cpool.tile([D + 1, m], fdt, tag=f"ws{l}",
                                            name=f"ws{l}")

            nc.sync.dma_start(idx_sb[:], idx_d[:])
            nc.sync.dma_start(code_sb[:], code_d[:])
            nc.sync.dma_start(scl2_sb[:], scl2_d[:])
            nc.sync.dma_start(iota_sb[:], iota_d[:])
            nc.sync.dma_start(ident_sb[:], ident_d[:])
            nc.sync.dma_start(hs[0][:], xsT_d[:])
            for l in range(3):
                nc.sync.dma_start(w_sb[f"wl{l}"][:], w_d[f"wl{l}"][:])
                nc.sync.dma_start(w_sb[f"ws{l}"][:], w_d[f"ws{l}"][:])
            nc.vector.memset(zrow_sb[:], 0.0)
            nc.vector.memset(zpad_sb[:], 0.0)
            nc.vector.memset(hs[1][D : D + 1, :], 1.0)
            nc.vector.memset(hs[2][D : D + 1, :], 1.0)
            # appended zero rows of the hext1 buffers
            nc.sync.dma_start(he1[0][H1TOT : H1TOT + 1, :], zrow_sb[:])
            nc.sync.dma_start(he1[1][H1TOT : H1TOT + 1, :], zrow_sb[:])

            for layer in range(3):
                if layer == 0:
                    srcA_t, srcB_t = xe1_d, xe2_d
                else:
                    srcA_t, srcB_t = he1[layer - 1], he2[layer - 1]
                hself = hs[layer]
                wl_t = w_sb[f"wl{layer}"]
                ws_t = w_sb[f"ws{layer}"]
                m_out = 1 if layer == 2 else D

                # software pipeline state: windows awaiting dense / transpose
                pend_dense = []  # (w, mean_sb)
                pend_tr = []     # (w, hn_sb)

                def do_dense(w, mean_sb):
                    y_ps = psC.tile([WIN, m_out], dt.float32, tag="ypsum")
                    nc.tensor.matmul(y_ps[:], mean_sb[:], wl_t[:],
                                     start=True, stop=False)
                    nc.tensor.matmul(y_ps[:],
                                     hself[:, w * WIN : (w + 1) * WIN],
                                     ws_t[:], start=False, stop=True)
                    if layer < 2:
                        hn_sb = spool.tile([WIN, D], fdt, tag="hn")
                        nc.scalar.activation(
                            hn_sb[:], y_ps[:],
                            mybir.ActivationFunctionType.Relu,
                        )
                        if w < NW1:
                            nc.sync.dma_start(
                                slab1_d[w * WIN : (w + 1) * WIN, 0:D], hn_sb[:]
                            )
                        else:
                            r0 = w * WIN - HALF
                            nc.sync.dma_start(
                                slab2_d[r0 : r0 + WIN, 0:D], hn_sb[:]
                            )
                        pend_tr.append((w, hn_sb))
                    else:
                        y_sb = spool.tile([WIN, 1], dt.float32, tag="ysb")
                        nc.scalar.activation(
                            y_sb[:], y_ps[:],
                            mybir.ActivationFunctionType.Relu,
                        )
                        nc.sync.dma_start(
                            out_d[w * WIN : (w + 1) * WIN, :], y_sb[:]
                        )

                def do_transpose(w, hn_sb):
                    t_ps = psB.tile([D, WIN], fdt, tag="tps", name="t_ps")
                    nc.tensor.transpose(t_ps[:], hn_sb[:], ident_sb[:])
                    nc.vector.tensor_copy(
                        hs[layer + 1][0:D, w * WIN : (w + 1) * WIN], t_ps[:]
                    )

                for bi, (w0, bw) in enumerate(batches):
                    gA = gpool.tile([128, bw * nch_a, ROW], fdt, tag="gA")
                    gB = gpool.tile([128, bw * nch_b, ROW], fdt, tag="gB")
                    # build the scaled one-hot on DVE: (iota == code) * scale
                    C = bw * NCH
                    oh_sb = ohpool.tile([128, C * 128], dt.bfloat16, tag="oh")
                    ovf = oh_sb[:].rearrange("p (c j) -> p c j", j=128)
                    ivb = iota_sb[:].unsqueeze(1).broadcast_to(
                        [128, C, 128]
                    )
                    cv = code_sb[:, w0 * NCH : w0 * NCH + C].unsqueeze(
                        2
                    ).broadcast_to([128, C, 128])
                    nc.vector.tensor_tensor(ovf, ivb, cv,
                                            mybir.AluOpType.is_equal)
                    numA = bw * S_A
                    numB = bw * S_B
                    a0 = w0 * S_A // 16
                    b0c = (T_A + w0 * S_B) // 16
                    hA = ((numA // 2) // 128) * 128
                    hB = ((numB // 2) // 128) * 128
                    hAc = hA // 128
                    hBc = hB // 128
                    nc.gpsimd.dma_gather(
                        gA[:, 0:hAc, :], srcA_t[:],
                        idx_sb[:, a0 : a0 + hA // 16],
                        hA, hA, ROW,
                        single_packet=False, queue_num=1,
                    )
                    nc.gpsimd.dma_gather(
                        gA[:, hAc:, :], srcA_t[:],
                        idx_sb[:, a0 + hA // 16 : a0 + numA // 16],
                        numA - hA, numA - hA, ROW,
                        single_packet=False, queue_num=2,
                    )
                    nc.gpsimd.dma_gather(
                        gB[:, 0:hBc, :], srcB_t[:],
                        idx_sb[:, b0c : b0c + hB // 16],
                        hB, hB, ROW,
                        single_packet=False, queue_num=3,
                    )
                    # batches right before a collective avoid queue 0, whose
                    # instruction would block the gpsimd engine and delay the
                    # collective dispatch
                    cc_next = (w0 + bw - 1 >= NW1 - 1 and w0 < NW1) or \
                        bi == len(batches) - 1
                    nc.gpsimd.dma_gather(
                        gB[:, hBc:, :], srcB_t[:],
                        idx_sb[:, b0c + hB // 16 : b0c + numB // 16],
                        numB - hB, numB - hB, ROW,
                        single_packet=False, queue_num=3 if cc_next else 0,
                    )

                    for wi in range(bw):
                        w = w0 + wi
                        win_ps = psA.tile([D, WIN], dt.float32, tag="winps")
                        for cc in range(NCH):
                            if cc < nch_a:
                                lhsT = gA[:, wi * nch_a + cc, 0:D]
                            else:
                                lhsT = gB[:, wi * nch_b + (cc - nch_a), 0:D]
                            oc = (wi * NCH + cc) * 128
                            nc.tensor.matmul(
                                win_ps[:], lhsT, oh_sb[:, oc : oc + 128],
                                start=(cc == 0), stop=(cc == NCH - 1),
                            )
                        mean_sb = spool.tile([D, WIN], fdt, tag="mean")
                        nc.vector.tensor_tensor(
                            mean_sb[:], win_ps[:],
                            scl2_sb[:, w * WIN : (w + 1) * WIN],
                            mybir.AluOpType.mult,
                        )
                        do_dense(w, mean_sb)
                        while pend_tr:
                            do_transpose(*pend_tr.pop(0))

                    # half1 AllGather needs windows 0-24 flushed
                    if layer < 2 and w0 + bw - 1 >= NW1 - 1 and w0 < NW1:
                        while pend_dense:
                            do_dense(*pend_dense.pop(0))
                        while pend_tr:
                            do_transpose(*pend_tr.pop(0))
                        nc.gpsimd.collective_compute(
                            "AllGather",
                            mybir.AluOpType.bypass,
                            replica_groups=[list(range(NCORES))],
                            ins=[slab1_d[:]],
                            outs=[he1[layer][0:H1TOT, :]],
                        )

                while pend_dense:
                    do_dense(*pend_dense.pop(0))
                while pend_tr:
                    do_transpose(*pend_tr.pop(0))
                if layer < 2:
                    nc.sync.dma_start(slab2_d[SLAB - HALF :, :], zpad_sb[:])
                    nc.gpsimd.collective_compute(
                        "AllGather",
                        mybir.AluOpType.bypass,
                        replica_groups=[list(range(NCORES))],
                        ins=[slab2_d[:]],
                        outs=[he2[layer][:]],
                    )

    nc.compile()
    return nc


def kernel(**inputs):
    x = np.asarray(inputs["x"], dtype=np.float32)
    edge_index = np.asarray(inputs["edge_index"])

    deg = np.bincount(np.asarray(edge_index[1], dtype=np.int64), minlength=N)
    scale = np.where(deg > 0, 1.0 / np.maximum(deg, 1), 0.0).astype(np.float32)

    nch_a, nch_b, xext1, xext2, cores = _pack(x, edge_index, scale)

    key = (nch_a, nch_b)
    if key not in _NC_CACHE:
        _NC_CACHE[key] = _build_nc(nch_a, nch_b)
    nc = _NC_CACHE[key]

    fdt = ml_dtypes.bfloat16
    ident = np.eye(WIN, dtype=fdt)

    common = {
        "xext1": xext1,
        "xext2": xext2,
        "ident": ident,
        "iota": np.tile(np.arange(WIN, dtype=np.float32), (WIN, 1)).astype(fdt),
    }
    for l in range(3):
        common[f"wl{l}"] = np.asarray(inputs[f"Wl{l}"]).astype(fdt)
        wse = np.concatenate(
            [
                np.asarray(inputs[f"Ws{l}"], np.float32),
                (np.asarray(inputs[f"bl{l}"], np.float32)
                 + np.asarray(inputs[f"bs{l}"], np.float32)).reshape(1, -1),
            ],
            axis=0,
        )
        common[f"ws{l}"] = wse.astype(fdt)

    in_maps = []
    for k in range(NCORES):
        m = dict(common)
        m.update(cores[k])
        m["idx"] = cores[k]["idx"]
        in_maps.append(m)

    from concourse.bass_utils import run_bass_kernel_spmd

    res = run_bass_kernel_spmd(nc, in_maps, core_ids=list(range(NCORES)))
    global LAST_RESULTS
    LAST_RESULTS = res
    outs = [np.asarray(res.results[k]["out"]).reshape(-1)[:SLAB]
            for k in range(NCORES)]
    return np.concatenate(outs).reshape(N, 1).astype(np.float32)


if __name__ == "__main__":
    pass


# revision 31
# speedup vs baseline: 1.0447x; 1.0447x over previous
"""Trainium2 Bass kernel for 3-layer GraphSAGE (nn_MCHCGraphSage).

Strategy (8 NeuronCores, SPMD single program):
  - Destination-sharded edges: core k owns dst nodes [k*6250, (k+1)*6250).
  - Node features stored in HBM as 256B rows in TWO half-spaces split by
    local row: half1 = slab rows [0, 3200), half2 = [3200, 6272).  Row
    addresses then fit int16 for the gpsimd dma_gather (25600 / 24576 rows).
  - Per batch of 4 dst windows, FOUR dma_gathers (half1/half2 x 2) on SWDGE
    queues 1,2,3,0: each queue uses its own Q7 core pair, so descriptor
    generation runs 4-way parallel (queue 0 dispatch blocks, so it goes
    last).
  - Aggregation: single-level one-hot matmul.  For each 128-slot chunk,
    matmul(win_ps[64, 128dst], lhsT=chunk[128slots, 64feat],
    rhs=oh[128slots, 128dst]) accumulates the *mean* directly: the host
    bakes 1/deg into the one-hot values.  Pad slots have all-zero one-hot
    rows, so no degree padding and no zero-fill matmuls are needed.
    One-hots stream from HBM per batch (double buffered).
  - Dense part per window, node-major: y = meanT.T @ Wl + hselfT.T @ Ws_ext
    (bias folded as an extra ones-row of hselfT), ReLU on ACT, PE-transpose
    to keep the feature-major self slab for the next layer.
  - Inter-layer redistribution: TWO partial AllGathers per layer (half1
    fires as soon as windows 0-24 are done, overlapping the rest of the
    layer; half2 at layer end), into per-layer-parity hext buffers.
"""

import os
import sys

import numpy as np

for _p in ("/opt/trn_rl_repo", "/root/.axon_site/_ro/trn_rl_repo"):
    if os.path.isdir(_p) and _p not in sys.path:
        sys.path.append(_p)

import ml_dtypes  # noqa: E402

N = 50000
D = 64
NCORES = 8
SLAB = 6250
PSLAB = 6272
WIN = 128
NW = PSLAB // WIN  # 49
HALF = 3200        # local rows in half1 (windows 0-24)
H2 = PSLAB - HALF  # 3072 (windows 25-48)
NW1 = HALF // WIN  # 25
H1TOT = NCORES * HALF   # 25600
H2TOT = NCORES * H2     # 24576
AZERO = H1TOT           # appended all-zero row in xext1/hext1
BZERO = 3050            # core-0 slab pad rows (local 6250) are always zero
BW = 4  # windows per gather batch

_NC_CACHE = {}
LAST_RESULTS = None  # test harness introspection (exec_time_ns, profile)


def _pack(x, edge_index, scale):
    """Host-side packing. Returns per-core dicts + structure constants."""
    src = np.asarray(edge_index[0], dtype=np.int64)
    dst = np.asarray(edge_index[1], dtype=np.int64)
    k_src = src // SLAB
    loc = src % SLAB
    isA_e = loc < HALF
    rowA_e = k_src * HALF + loc
    rowB_e = k_src * H2 + (loc - HALF)

    # pass 1: section sizes
    nch_a = 0
    nch_b = 0
    per_core = []
    for k in range(NCORES):
        sel = (dst >= k * SLAB) & (dst < (k + 1) * SLAB)
        d_k = dst[sel] - k * SLAB
        isA = isA_e[sel]
        row_k = np.where(isA, rowA_e[sel], rowB_e[sel])
        degA = np.bincount(d_k[isA], minlength=PSLAB)
        degB = np.bincount(d_k[~isA], minlength=PSLAB)
        wA = degA.reshape(NW, WIN).sum(1).max()
        wB = degB.reshape(NW, WIN).sum(1).max()
        nch_a = max(nch_a, (int(wA) + 127) // 128)
        nch_b = max(nch_b, (int(wB) + 127) // 128)
        per_core.append((d_k, row_k, isA, degA, degB))

    S_A = nch_a * 128
    S_B = nch_b * 128
    NCH = nch_a + nch_b
    T_A = NW * S_A
    T_B = NW * S_B
    fdt = ml_dtypes.bfloat16
    ROW = 128

    # node features in the two half-spaces (256B rows, 64 used)
    nodes = np.arange(N)
    nloc = nodes % SLAB
    nk = nodes // SLAB
    xext1 = np.zeros((H1TOT + 1, ROW), dtype=fdt)
    xext2 = np.zeros((H2TOT, ROW), dtype=fdt)
    m1 = nloc < HALF
    xext1[(nk * HALF + nloc)[m1], :D] = x[m1].astype(fdt)
    xext2[(nk * H2 + nloc - HALF)[~m1], :D] = x[~m1].astype(fdt)

    cores = []
    for k in range(NCORES):
        d_k, row_k, isA, degA, degB = per_core[k]
        offA = degA.reshape(NW, WIN)
        offA = (np.cumsum(offA, 1) - offA).reshape(-1)
        offB = degB.reshape(NW, WIN)
        offB = (np.cumsum(offB, 1) - offB).reshape(-1)

        def build(mask, deg, off, S, padval):
            e_d = d_k[mask]
            e_r = row_k[mask]
            order = np.argsort(e_d, kind="stable")
            d_s = e_d[order]
            r_s = e_r[order]
            start = np.concatenate([[0], np.cumsum(deg)])[:-1]
            rank = np.arange(len(d_s)) - start[d_s]
            pos = (d_s // WIN) * S + off[d_s] + rank
            stream = np.full(NW * S, padval, dtype=np.int64)
            stream[pos] = r_s
            return stream, pos, d_s

        streamA, posA, dA = build(isA, degA, offA, S_A, AZERO)
        streamB, posB, dB = build(~isA, degB, offB, S_B, BZERO)
        assert streamA.max() <= AZERO and streamB.max() < H2TOT
        assert streamA.min() >= 0 and streamB.min() >= 0

        # per-slot dst code + 1/deg scale; the device builds the one-hot
        # rhs per chunk as (iota == code) * scale on the DVE.
        code = np.full((128, NW * NCH), 999.0, dtype=fdt)
        sc_k = scale[k * SLAB : (k + 1) * SLAB]
        scp = np.zeros(PSLAB, dtype=np.float32)
        scp[:SLAB] = sc_k
        for pos, d_s, cc0, S in ((posA, dA, 0, S_A), (posB, dB, nch_a, S_B)):
            w = pos // S
            r = pos % S
            cc = cc0 + r // 128
            prow = r % 128
            code[prow, w * NCH + cc] = (d_s % WIN).astype(np.float32)
        scl2 = np.tile(scp.astype(np.float32), (D, 1)).astype(fdt)

        stream = np.concatenate([streamA, streamB]).astype(np.int16)
        idx16 = stream.reshape(-1, 16).T.copy()  # [16, T/16]
        idx = np.tile(idx16, (8, 1))  # replicate for 8 gpsimd cores

        xselfT = np.zeros((D + 1, PSLAB), dtype=fdt)
        xselfT[:D, :SLAB] = x[k * SLAB : (k + 1) * SLAB].T.astype(fdt)
        xselfT[D, :] = 1.0  # bias row

        cores.append({"idx": idx, "code": code, "scl2": scl2, "xselfT": xselfT})

    return nch_a, nch_b, xext1, xext2, cores


def _build_nc(nch_a, nch_b):
    import concourse.bacc as bacc
    import concourse.tile as tile
    import concourse.mybir as mybir

    dt = mybir.dt
    fdt = dt.bfloat16
    ROW = 128
    NCH = nch_a + nch_b
    S_A = nch_a * 128
    S_B = nch_b * 128
    T_A = NW * S_A
    T_B = NW * S_B

    nqueues = int(os.environ.get("SAGE_QUEUES", "4"))
    nc = bacc.Bacc(None, num_devices=NCORES, num_swdge_queues=nqueues)

    xe1_d = nc.dram_tensor("xext1", [H1TOT + 1, ROW], fdt, kind="ExternalInput")
    xe2_d = nc.dram_tensor("xext2", [H2TOT, ROW], fdt, kind="ExternalInput")
    idx_d = nc.dram_tensor(
        "idx", [128, (T_A + T_B) // 16], dt.int16, kind="ExternalInput"
    )
    code_d = nc.dram_tensor(
        "code", [128, NW * NCH], dt.bfloat16, kind="ExternalInput"
    )
    scl2_d = nc.dram_tensor(
        "scl2", [D, PSLAB], dt.bfloat16, kind="ExternalInput"
    )
    iota_d = nc.dram_tensor("iota", [128, 128], dt.bfloat16, kind="ExternalInput")
    xsT_d = nc.dram_tensor("xselfT", [D + 1, PSLAB], fdt, kind="ExternalInput")
    ident_d = nc.dram_tensor("ident", [WIN, WIN], fdt, kind="ExternalInput")
    w_d = {}
    for l, m in ((0, D), (1, D), (2, 1)):
        w_d[f"wl{l}"] = nc.dram_tensor(f"wl{l}", [D, m], fdt, kind="ExternalInput")
        w_d[f"ws{l}"] = nc.dram_tensor(
            f"ws{l}", [D + 1, m], fdt, kind="ExternalInput"
        )
    out_d = nc.dram_tensor("out", [PSLAB, 1], dt.float32, kind="ExternalOutput")

    he1 = [nc.dram_tensor(f"hext1{p}", [H1TOT + 1, ROW], fdt, addr_space="Shared")
           for p in "ab"]
    he2 = [nc.dram_tensor(f"hext2{p}", [H2TOT, ROW], fdt, addr_space="Shared")
           for p in "ab"]
    slab1_d = nc.dram_tensor("slab1", [HALF, ROW], fdt)
    slab2_d = nc.dram_tensor("slab2", [H2, ROW], fdt)

    batches = []
    w0 = 0
    while w0 < NW:
        bw = min(BW, NW - w0)
        batches.append((w0, bw))
        w0 += bw

    with tile.TileContext(nc) as tc:
        with (
            tc.tile_pool(name="const", bufs=1) as cpool,
            tc.tile_pool(name="gpool", bufs=2) as gpool,
            tc.tile_pool(name="ohpool", bufs=2) as ohpool,
            tc.tile_pool(name="spool", bufs=4) as spool,
            tc.tile_pool(name="psA", bufs=4, space="PSUM") as psA,
            tc.tile_pool(name="psB", bufs=2, space="PSUM") as psB,
            tc.tile_pool(name="psC", bufs=2, space="PSUM") as psC,
        ):
            idx_sb = cpool.tile([128, (T_A + T_B) // 16], dt.int16, tag="idx")
            code_sb = cpool.tile([128, NW * NCH], dt.bfloat16, tag="code")
            scl2_sb = cpool.tile([D, PSLAB], dt.bfloat16, tag="scl2")
            iota_sb = cpool.tile([128, 128], dt.bfloat16, tag="iota")
            ident_sb = cpool.tile([WIN, WIN], fdt, tag="ident")
            zrow_sb = cpool.tile([1, ROW], fdt, tag="zrow")
            zpad_sb = cpool.tile([PSLAB - SLAB, ROW], fdt, tag="zpad")
            hs = [cpool.tile([D + 1, PSLAB], fdt, tag=f"hs{i}", name=f"hs{i}")
                  for i in range(3)]
            w_sb = {}
            for l, m in ((0, D), (1, D), (2, 1)):
                w_sb[f"wl{l}"] = cpool.tile([D, m], fdt, tag=f"wl{l}",
                                            name=f"wl{l}")
                w_sb[f"ws{l}"] = cpool.tile([D + 1, m], fdt, tag=f"ws{l}",
                                            name=f"ws{l}")

            nc.sync.dma_start(idx_sb[:], idx_d[:])
            nc.sync.dma_start(code_sb[:], code_d[:])
            nc.sync.dma_start(scl2_sb[:], scl2_d[:])
            nc.sync.dma_start(iota_sb[:], iota_d[:])
            nc.sync.dma_start(ident_sb[:], ident_d[:])
            nc.sync.dma_start(hs[0][:], xsT_d[:])
            for l in range(3):
                nc.sync.dma_start(w_sb[f"wl{l}"][:], w_d[f"wl{l}"][:])
                nc.sync.dma_start(w_sb[f"ws{l}"][:], w_d[f"ws{l}"][:])
            nc.vector.memset(zrow_sb[:], 0.0)
            nc.vector.memset(zpad_sb[:], 0.0)
            nc.vector.memset(hs[1][D : D + 1, :], 1.0)
            nc.vector.memset(hs[2][D : D + 1, :], 1.0)
            # appended zero rows of the hext1 buffers
            nc.sync.dma_start(he1[0][H1TOT : H1TOT + 1, :], zrow_sb[:])
            nc.sync.dma_start(he1[1][H1TOT : H1TOT + 1, :], zrow_sb[:])

            hoistA = []  # prefetched gA tiles for the next layer's batches
            for layer in range(3):
                if layer == 0:
                    srcA_t, srcB_t = xe1_d, xe2_d
                else:
                    srcA_t, srcB_t = he1[layer - 1], he2[layer - 1]
                hself = hs[layer]
                wl_t = w_sb[f"wl{layer}"]
                ws_t = w_sb[f"ws{layer}"]
                m_out = 1 if layer == 2 else D

                # software pipeline state: windows awaiting dense / transpose
                pend_dense = []  # (w, mean_sb)
                pend_tr = []     # (w, hn_sb)

                def do_dense(w, mean_sb):
                    y_ps = psC.tile([WIN, m_out], dt.float32, tag="ypsum")
                    nc.tensor.matmul(y_ps[:], mean_sb[:], wl_t[:],
                                     start=True, stop=False)
                    nc.tensor.matmul(y_ps[:],
                                     hself[:, w * WIN : (w + 1) * WIN],
                                     ws_t[:], start=False, stop=True)
                    if layer < 2:
                        hn_sb = spool.tile([WIN, D], fdt, tag="hn")
                        nc.scalar.activation(
                            hn_sb[:], y_ps[:],
                            mybir.ActivationFunctionType.Relu,
                        )
                        if w < NW1:
                            nc.sync.dma_start(
                                slab1_d[w * WIN : (w + 1) * WIN, 0:D], hn_sb[:]
                            )
                        else:
                            r0 = w * WIN - HALF
                            nc.sync.dma_start(
                                slab2_d[r0 : r0 + WIN, 0:D], hn_sb[:]
                            )
                        pend_tr.append((w, hn_sb))
                    else:
                        y_sb = spool.tile([WIN, 1], dt.float32, tag="ysb")
                        nc.scalar.activation(
                            y_sb[:], y_ps[:],
                            mybir.ActivationFunctionType.Relu,
                        )
                        nc.sync.dma_start(
                            out_d[w * WIN : (w + 1) * WIN, :], y_sb[:]
                        )

                def do_transpose(w, hn_sb):
                    t_ps = psB.tile([D, WIN], fdt, tag="tps", name="t_ps")
                    nc.tensor.transpose(t_ps[:], hn_sb[:], ident_sb[:])
                    nc.vector.tensor_copy(
                        hs[layer + 1][0:D, w * WIN : (w + 1) * WIN], t_ps[:]
                    )

                for bi, (w0, bw) in enumerate(batches):
                    if bi < len(hoistA):
                        gA = hoistA[bi]
                    else:
                        gA = gpool.tile([128, bw * nch_a, ROW], fdt, tag="gA")
                    gB = gpool.tile([128, bw * nch_b, ROW], fdt, tag="gB")
                    # build the scaled one-hot on DVE: (iota == code) * scale
                    C = bw * NCH
                    oh_sb = ohpool.tile([128, C * 128], dt.bfloat16, tag="oh")
                    ovf = oh_sb[:].rearrange("p (c j) -> p c j", j=128)
                    ivb = iota_sb[:].unsqueeze(1).broadcast_to(
                        [128, C, 128]
                    )
                    cv = code_sb[:, w0 * NCH : w0 * NCH + C].unsqueeze(
                        2
                    ).broadcast_to([128, C, 128])
                    nc.vector.tensor_tensor(ovf, ivb, cv,
                                            mybir.AluOpType.is_equal)
                    numA = bw * S_A
                    numB = bw * S_B
                    a0 = w0 * S_A // 16
                    b0c = (T_A + w0 * S_B) // 16
                    hA = ((numA // 2) // 128) * 128
                    hB = ((numB // 2) // 128) * 128
                    hAc = hA // 128
                    hBc = hB // 128
                    if bi >= len(hoistA):
                        nc.gpsimd.dma_gather(
                            gA[:, 0:hAc, :], srcA_t[:],
                            idx_sb[:, a0 : a0 + hA // 16],
                            hA, hA, ROW,
                            single_packet=False, queue_num=1,
                        )
                        nc.gpsimd.dma_gather(
                            gA[:, hAc:, :], srcA_t[:],
                            idx_sb[:, a0 + hA // 16 : a0 + numA // 16],
                            numA - hA, numA - hA, ROW,
                            single_packet=False, queue_num=2,
                        )
                    nc.gpsimd.dma_gather(
                        gB[:, 0:hBc, :], srcB_t[:],
                        idx_sb[:, b0c : b0c + hB // 16],
                        hB, hB, ROW,
                        single_packet=False, queue_num=3,
                    )
                    # batches right before a collective avoid queue 0, whose
                    # instruction would block the gpsimd engine and delay the
                    # collective dispatch
                    cc_next = (w0 + bw - 1 >= NW1 - 1 and w0 < NW1) or \
                        bi == len(batches) - 1
                    nc.gpsimd.dma_gather(
                        gB[:, hBc:, :], srcB_t[:],
                        idx_sb[:, b0c + hB // 16 : b0c + numB // 16],
                        numB - hB, numB - hB, ROW,
                        single_packet=False, queue_num=3 if cc_next else 0,
                    )

                    for wi in range(bw):
                        w = w0 + wi
                        win_ps = psA.tile([D, WIN], dt.float32, tag="winps")
                        for cc in range(NCH):
                            if cc < nch_a:
                                lhsT = gA[:, wi * nch_a + cc, 0:D]
                            else:
                                lhsT = gB[:, wi * nch_b + (cc - nch_a), 0:D]
                            oc = (wi * NCH + cc) * 128
                            nc.tensor.matmul(
                                win_ps[:], lhsT, oh_sb[:, oc : oc + 128],
                                start=(cc == 0), stop=(cc == NCH - 1),
                            )
                        mean_sb = spool.tile([D, WIN], fdt, tag="mean")
                        nc.vector.tensor_tensor(
                            mean_sb[:], win_ps[:],
                            scl2_sb[:, w * WIN : (w + 1) * WIN],
                            mybir.AluOpType.mult,
                        )
                        do_dense(w, mean_sb)
                        while pend_tr:
                            do_transpose(*pend_tr.pop(0))

                    # half1 AllGather needs windows 0-24 flushed
                    if layer < 2 and w0 + bw - 1 >= NW1 - 1 and w0 < NW1:
                        while pend_dense:
                            do_dense(*pend_dense.pop(0))
                        while pend_tr:
                            do_transpose(*pend_tr.pop(0))
                        nc.gpsimd.collective_compute(
                            "AllGather",
                            mybir.AluOpType.bypass,
                            replica_groups=[list(range(NCORES))],
                            ins=[slab1_d[:]],
                            outs=[he1[layer][0:H1TOT, :]],
                        )

                while pend_dense:
                    do_dense(*pend_dense.pop(0))
                while pend_tr:
                    do_transpose(*pend_tr.pop(0))
                hoist_next = []
                if layer < 2:
                    # prefetch the next layer's A-side gathers for the first
                    # batches: they only need cc1 (done mid-layer), so issue
                    # them before cc2 blocks the layer boundary
                    for (w0h, bwh) in batches[:2]:
                        gAh = gpool.tile([128, bwh * nch_a, ROW], fdt,
                                         tag="gA")
                        numAh = bwh * S_A
                        a0h = w0h * S_A // 16
                        hAh = ((numAh // 2) // 128) * 128
                        hAch = hAh // 128
                        nc.gpsimd.dma_gather(
                            gAh[:, 0:hAch, :], he1[layer][:],
                            idx_sb[:, a0h : a0h + hAh // 16],
                            hAh, hAh, ROW,
                            single_packet=False, queue_num=1,
                        )
                        nc.gpsimd.dma_gather(
                            gAh[:, hAch:, :], he1[layer][:],
                            idx_sb[:, a0h + hAh // 16 : a0h + numAh // 16],
                            numAh - hAh, numAh - hAh, ROW,
                            single_packet=False, queue_num=2,
                        )
                        hoist_next.append(gAh)
                    nc.sync.dma_start(slab2_d[SLAB - HALF :, :], zpad_sb[:])
                    nc.gpsimd.collective_compute(
                        "AllGather",
                        mybir.AluOpType.bypass,
                        replica_groups=[list(range(NCORES))],
                        ins=[slab2_d[:]],
                        outs=[he2[layer][:]],
                    )
                hoistA = hoist_next

    nc.compile()
    return nc


def kernel(**inputs):
    x = np.asarray(inputs["x"], dtype=np.float32)
    edge_index = np.asarray(inputs["edge_index"])

    deg = np.bincount(np.asarray(edge_index[1], dtype=np.int64), minlength=N)
    scale = np.where(deg > 0, 1.0 / np.maximum(deg, 1), 0.0).astype(np.float32)

    nch_a, nch_b, xext1, xext2, cores = _pack(x, edge_index, scale)

    key = (nch_a, nch_b)
    if key not in _NC_CACHE:
        _NC_CACHE[key] = _build_nc(nch_a, nch_b)
    nc = _NC_CACHE[key]

    fdt = ml_dtypes.bfloat16
    ident = np.eye(WIN, dtype=fdt)

    common = {
        "xext1": xext1,
        "xext2": xext2,
        "ident": ident,
        "iota": np.tile(np.arange(WIN, dtype=np.float32), (WIN, 1)).astype(fdt),
    }
    for l in range(3):
        common[f"wl{l}"] = np.asarray(inputs[f"Wl{l}"]).astype(fdt)
        wse = np.concatenate(
            [
                np.asarray(inputs[f"Ws{l}"], np.float32),
                (np.asarray(inputs[f"bl{l}"], np.float32)
                 + np.asarray(inputs[f"bs{l}"], np.float32)).reshape(1, -1),
            ],
            axis=0,
        )
        common[f"ws{l}"] = wse.astype(fdt)

    in_maps = []
    for k in range(NCORES):
        m = dict(common)
        m.update(cores[k])
        m["idx"] = cores[k]["idx"]
        in_maps.append(m)

    from concourse.bass_utils import run_bass_kernel_spmd

    res = run_bass_kernel_spmd(nc, in_maps, core_ids=list(range(NCORES)))
    global LAST_RESULTS
    LAST_RESULTS = res
    outs = [np.asarray(res.results[k]["out"]).reshape(-1)[:SLAB]
            for k in range(NCORES)]
    return np.concatenate(outs).reshape(N, 1).astype(np.float32)


if __name__ == "__main__":
    pass
